# Initial kernel scaffold
#
"""Multi-head self-attention (B=2, S=1024, D=1024, H=16) on 8 TRN2 NeuronCores.

Sharding: pure tensor-parallel over heads (2 heads/core, both batch elements),
Megatron-style. Per core:
  1. QKV^T projection with f32r matmuls: Q^T, K^T (head dims on partitions) and
     V^T, which is PE-transposed to V-natural layout with an appended ones
     column (yields softmax denominators for free during attn@V).
  2. scores^T = K_h Q_h^T per (head, batch) with k-tokens on partitions;
     exp on ACT (no max subtraction -- logits are O(1) by construction);
     attn@V accumulated over k-tiles -> unnormalized attnout^T + denominator row.
  3. Normalize via reciprocal + PE broadcast, one 8-rank AllToAll converts
     head-sharding to token-sharding, then the output projection runs with the
     full W_out on each core for its 256-token shard.
Host assembles the 8 [1024 e, 256 s] shards into [2, 1024, 1024].
"""
import sys

sys.path.insert(0, "/opt/trn_rl_repo")

import numpy as np

B, S, D, H = 2, 1024, 1024, 16
DH = D // H
N_CORES = 8
HEADS_PER_CORE = H // N_CORES  # 2
SCALE = 1.0 / float(np.sqrt(DH))
NT = S // 128   # 8 token tiles per batch
NDT = D // 128  # 8 d tiles

_nc_cache = None


def _build_nc():
    global _nc_cache
    if _nc_cache is not None:
        return _nc_cache

    import concourse.bacc as bacc
    import concourse.mybir as mybir
    import concourse.tile as tile

    F32 = mybir.dt.float32
    F32R = mybir.dt.float32r
    EXP = mybir.ActivationFunctionType.Exp

    nc = bacc.Bacc(None, target_bir_lowering=False)

    xT = nc.dram_tensor("xT", [B, D, S], F32, kind="ExternalInput")
    wqkvT = nc.dram_tensor("wqkvT", [D, 384], F32, kind="ExternalInput")
    woT = nc.dram_tensor("woT", [D, D], F32, kind="ExternalInput")
    ident = nc.dram_tensor("ident", [128, 128], F32, kind="ExternalInput")
    onesT = nc.dram_tensor("onesT", [128, 2], F32, kind="ExternalInput")
    outT = nc.dram_tensor("outT", [D, S * B // N_CORES], F32, kind="ExternalOutput")

    a2a_in = nc.dram_tensor("a2a_in", [D, 256], F32)
    a2a_out = nc.dram_tensor("a2a_out", [D, 256], F32, addr_space="Shared")

    with tile.TileContext(nc) as tc, nc.allow_low_precision(reason="f32r attention"):
        with (
            tc.tile_pool(name="const", bufs=1) as const_pool,
            tc.tile_pool(name="wk", bufs=NDT) as wk_pool,
            tc.tile_pool(name="qk", bufs=4) as qk_pool,
            tc.tile_pool(name="v65", bufs=B * NT) as v65_pool,
        ):
            idt = const_pool.tile([128, 128], F32R, tag="ident")
            nc.sync.dma_start(idt[:], ident[:].bitcast(F32R))
            ones_t = const_pool.tile([128, 2], F32R, tag="ones")
            nc.sync.dma_start(ones_t[:], onesT[:].bitcast(F32R))

            wk = []
            for dt in range(NDT):
                w = wk_pool.tile([128, 384], F32R, tag="wk")
                nc.sync.dma_start(w[:], wqkvT[dt * 128:(dt + 1) * 128, :].bitcast(F32R))
                wk.append(w)

            # qt[b], kt[b]: [128 (2 heads x 64 qk-dims), 1024 tokens] f32r
            qt = [qk_pool.tile([128, S], F32R, tag="qk") for _ in range(B)]
            kt = [qk_pool.tile([128, S], F32R, tag="qk") for _ in range(B)]
            # v65[b][tb]: [128 tokens, 130] = [V_h0 | ones | V_h1 | ones]
            v65 = [[v65_pool.tile([128, 2 * (DH + 1)], F32R, tag="v65")
                    for _ in range(NT)] for _ in range(B)]

            with (
                tc.tile_pool(name="xt", bufs=B * NDT) as xt_pool,
                tc.tile_pool(name="vt", bufs=B) as vt_pool,
                tc.tile_pool(name="psB", bufs=2, space="PSUM") as psB_pool,
                tc.tile_pool(name="psT", bufs=2, space="PSUM") as psT_pool,
            ):
                xt = []
                for b in range(B):
                    row = []
                    for dt in range(NDT):
                        t = xt_pool.tile([128, S], F32R, tag="xt")
                        nc.sync.dma_start(
                            t[:], xT[b, dt * 128:(dt + 1) * 128, :].bitcast(F32R)
                        )
                        row.append(t)
                    xt.append(row)

                # ---- QKV^T: psum[qk-dims 128, tokens 1024], accumulate over d ----
                vt = []
                for b in range(B):
                    for part, dst in ((0, qt[b]), (1, kt[b])):
                        ps = psB_pool.tile([128, S], F32, tag="psB")
                        for dt in range(NDT):
                            for nb in range(2):
                                nc.tensor.matmul(
                                    ps[:, nb * 512:(nb + 1) * 512],
                                    wk[dt][:, part * 128:(part + 1) * 128],
                                    xt[b][dt][:, nb * 512:(nb + 1) * 512],
                                    start=(dt == 0),
                                    stop=(dt == NDT - 1),
                                )
                        nc.vector.tensor_copy(dst[:], ps[:].bitcast(F32R))
                    # V^T for this batch
                    ps = psB_pool.tile([128, S], F32, tag="psB")
                    for dt in range(NDT):
                        for nb in range(2):
                            nc.tensor.matmul(
                                ps[:, nb * 512:(nb + 1) * 512],
                                wk[dt][:, 256:384],
                                xt[b][dt][:, nb * 512:(nb + 1) * 512],
                                start=(dt == 0),
                                stop=(dt == NDT - 1),
                            )
                    v = vt_pool.tile([128, S], F32R, tag="vt")
                    nc.vector.tensor_copy(v[:], ps[:].bitcast(F32R))
                    vt.append(v)

                # ---- V^T -> V natural tiles with ones columns ----
                for b in range(B):
                    for tb in range(NT):
                        pst = psT_pool.tile([128, 128], F32R, tag="psT")
                        nc.tensor.transpose(
                            pst[:], vt[b][:, tb * 128:(tb + 1) * 128], idt[:]
                        )
                        dst = v65[b][tb]
                        for hl in range(2):
                            nc.vector.tensor_copy(
                                dst[:, hl * 65:hl * 65 + 64],
                                pst[:, hl * 64:(hl + 1) * 64],
                            )
                            nc.vector.tensor_copy(
                                dst[:, hl * 65 + 64:hl * 65 + 65],
                                ones_t[:, hl:hl + 1],
                            )

            # ---- attention per (batch, local head) ----
            with (
                tc.tile_pool(name="wo", bufs=NDT) as wo_pool,
                tc.tile_pool(name="expt", bufs=4) as exp_pool,
                tc.tile_pool(name="att", bufs=2) as att_pool,
                tc.tile_pool(name="rcp", bufs=2) as rcp_pool,
                tc.tile_pool(name="bcs", bufs=2) as bcs_pool,
                tc.tile_pool(name="af", bufs=NDT) as af_pool,
                tc.tile_pool(name="osb", bufs=2) as osb_pool,
                tc.tile_pool(name="psC", bufs=2, space="PSUM") as psC_pool,
                tc.tile_pool(name="psA", bufs=2, space="PSUM") as psA_pool,
            ):
                wo = []
                for dt in range(NDT):
                    w = wo_pool.tile([128, D], F32R, tag="wo")
                    nc.sync.dma_start(w[:], woT[dt * 128:(dt + 1) * 128, :].bitcast(F32R))
                    wo.append(w)

                att = [att_pool.tile([64, B * S], F32, tag="att") for _ in range(2)]

                for b in range(B):
                    for hl in range(2):
                        p0 = hl * 64
                        pav = psA_pool.tile([65, S], F32, tag="psA")
                        for kb in range(NT):
                            psc = psC_pool.tile([128, S], F32, tag="psC")
                            for nb in range(2):
                                nc.tensor.matmul(
                                    psc[:, nb * 512:(nb + 1) * 512],
                                    kt[b][p0:p0 + 64, kb * 128:(kb + 1) * 128],
                                    qt[b][p0:p0 + 64, nb * 512:(nb + 1) * 512],
                                    start=True,
                                    stop=True,
                                )
                            et = exp_pool.tile([128, S], F32R, tag="expt")
                            nc.scalar.activation(et[:], psc[:], EXP)
                            for nb in range(2):
                                nc.tensor.matmul(
                                    pav[:, nb * 512:(nb + 1) * 512],
                                    v65[b][kb][:, hl * 65:(hl + 1) * 65],
                                    et[:, nb * 512:(nb + 1) * 512],
                                    start=(kb == 0),
                                    stop=(kb == NT - 1),
                                )
                        # normalize: recip of denominator row, PE-broadcast, multiply
                        rc = rcp_pool.tile([65, S], F32R, tag="rcp")
                        nc.vector.reciprocal(rc[64:65, :], pav[64:65, :].bitcast(F32R))
                        pbc = psA_pool.tile([64, S], F32, tag="psA")
                        for nb in range(2):
                            nc.tensor.matmul(
                                pbc[:, nb * 512:(nb + 1) * 512],
                                ones_t[64:65, 0:1].broadcast(1, 64)
                                if hasattr(ones_t[64:65, 0:1], "broadcast")
                                else ones_t[64:65, :1],
                                rc[64:65, nb * 512:(nb + 1) * 512],
                                start=True,
                                stop=True,
                            )
                        bc = bcs_pool.tile([64, S], F32, tag="bcs")
                        nc.vector.tensor_copy(bc[:], pbc[:])
                        nc.vector.tensor_mul(
                            att[hl][:, b * S:(b + 1) * S], pav[:64, :], bc[:]
                        )

                # ---- AllToAll: head-sharded -> token-sharded ----
                for j in range(N_CORES):
                    src = att[0] if True else None
                    col = (j // 4) * S + (j % 4) * 256
                    for hl in range(2):
                        nc.sync.dma_start(
                            a2a_in[j * 128 + hl * 64:j * 128 + (hl + 1) * 64, :],
                            att[hl][:, col:col + 256],
                        )
                nc.gpsimd.collective_compute(
                    "AllToAll",
                    mybir.AluOpType.bypass,
                    replica_groups=[list(range(N_CORES))],
                    ins=[a2a_in[:]],
                    outs=[a2a_out[:]],
                )
                af = []
                for dt in range(NDT):
                    t = af_pool.tile([128, 256], F32R, tag="af")
                    nc.sync.dma_start(
                        t[:], a2a_out[dt * 128:(dt + 1) * 128, :].bitcast(F32R)
                    )
                    af.append(t)

                # ---- output projection: full W_out, 256-token shard ----
                for eb in range(NDT):
                    po = psC_pool.tile([128, 256], F32, tag="psC")
                    for dt in range(NDT):
                        nc.tensor.matmul(
                            po[:],
                            wo[dt][:, eb * 128:(eb + 1) * 128],
                            af[dt][:],
                            start=(dt == 0),
                            stop=(dt == NDT - 1),
                        )
                    ot = osb_pool.tile([128, 256], F32, tag="osb")
                    nc.vector.tensor_copy(ot[:], po[:])
                    nc.sync.dma_start(outT[eb * 128:(eb + 1) * 128, :], ot[:])

    nc.finalize()
    _nc_cache = nc
    return nc


def prep_inputs(x, W_qkv, W_out):
    """Build per-core input maps (numpy only)."""
    x = np.asarray(x, dtype=np.float32)
    W_qkv = np.asarray(W_qkv, dtype=np.float32)
    W_out = np.asarray(W_out, dtype=np.float32)

    xT = np.ascontiguousarray(x.transpose(0, 2, 1))  # [B, D, S]
    woT = np.ascontiguousarray(W_out.T)              # [D, D]
    ident = np.eye(128, dtype=np.float32)
    onesT = np.ones((128, 2), dtype=np.float32)

    Wr = W_qkv.reshape(3, H, DH, D)
    in_maps = []
    for c in range(N_CORES):
        hs = slice(2 * c, 2 * c + 2)
        wq = Wr[0, hs].reshape(128, D) * SCALE
        wk = Wr[1, hs].reshape(128, D)
        wv = Wr[2, hs].reshape(128, D)
        wqkvT = np.ascontiguousarray(np.concatenate([wq, wk, wv], 0).T)  # [D, 384]
        in_maps.append({
            "xT": xT,
            "wqkvT": wqkvT,
            "woT": woT,
            "ident": ident,
            "onesT": onesT,
        })
    return in_maps


def assemble(results):
    out = np.empty((B, S, D), dtype=np.float32)
    for c in range(N_CORES):
        b, r = c // 4, c % 4
        out[b, r * 256:(r + 1) * 256, :] = results[c]["outT"].T
    return out


def kernel(x, W_qkv, W_out):
    from concourse.bass_utils import run_bass_kernel_spmd

    nc = _build_nc()
    in_maps = prep_inputs(x, W_qkv, W_out)
    res = run_bass_kernel_spmd(nc, in_maps, list(range(N_CORES)))
    return assemble(res.results)


# revision 6
# speedup vs baseline: 1.2144x; 1.2144x over previous
"""Multi-head self-attention (B=2, S=1024, D=1024, H=16) on 8 TRN2 NeuronCores.

Sharding: tensor-parallel over heads (2 heads/core, both batch elements),
Megatron-style. Per core:
  1. QKV^T projection (bf16 matmuls): Q^T, K^T (head dims on partitions) and
     V^T, which is PE-transposed to V-natural layout with an appended ones
     column (yields softmax denominators for free during attn@V).
  2. scores^T = K_h Q_h^T per (head, batch) with k-tokens on partitions;
     exp on ACT (no max subtraction -- logits are O(1) by construction);
     attn@V accumulated over k-tiles -> unnormalized attnout^T + denom row.
  3. Normalize via fp32r reciprocal + PE broadcast, one 8-rank AllToAll turns
     head-sharding into token-sharding, then the output projection runs with
     the full W_out on each core for its 256-token shard.
Host assembles the 8 [1024 e, 256 s] shards into [2, 1024, 1024].
"""
import sys

sys.path.insert(0, "/opt/trn_rl_repo")

import numpy as np
import ml_dtypes

B, S, D, H = 2, 1024, 1024, 16
DH = D // H
N_CORES = 8
SCALE = 1.0 / float(np.sqrt(DH))
NT = S // 128   # token tiles per batch
NDT = D // 128  # d tiles

BF16 = ml_dtypes.bfloat16

_nc_cache = {}


def _build_nc(iters=1):
    if iters in _nc_cache:
        return _nc_cache[iters]

    import concourse.bacc as bacc
    import concourse.mybir as mybir
    import concourse.tile as tile

    F32 = mybir.dt.float32
    F32R = mybir.dt.float32r
    BF = mybir.dt.bfloat16
    EXP = mybir.ActivationFunctionType.Exp

    nc = bacc.Bacc(None, target_bir_lowering=False)

    xT = nc.dram_tensor("xT", [B, D, S], BF, kind="ExternalInput")
    wqkvT = nc.dram_tensor("wqkvT", [D, 384], BF, kind="ExternalInput")
    woT = nc.dram_tensor("woT", [D, D], BF, kind="ExternalInput")
    ident = nc.dram_tensor("ident", [128, 128], BF, kind="ExternalInput")
    onesB = nc.dram_tensor("onesB", [128, 2], BF, kind="ExternalInput")
    onesF = nc.dram_tensor("onesF", [128, 66], F32, kind="ExternalInput")
    outT = nc.dram_tensor("outT", [D, S * B // N_CORES], F32, kind="ExternalOutput")

    a2a_in = [nc.dram_tensor(f"a2a_in{i}", [D, 256], BF) for i in range(iters)]
    a2a_out = [nc.dram_tensor(f"a2a_out{i}", [D, 256], BF) for i in range(iters)]

    with tile.TileContext(nc) as tc, nc.allow_low_precision(reason="bf16 attention"):
      for it in range(iters):
        with (
            tc.tile_pool(name="const", bufs=1) as const_pool,
            tc.tile_pool(name="wk", bufs=NDT) as wk_pool,
            tc.tile_pool(name="qk", bufs=4) as qk_pool,
            tc.tile_pool(name="v65", bufs=B * NT) as v65_pool,
        ):
            idt = const_pool.tile([128, 128], BF, tag="ident")
            nc.sync.dma_start(idt[:], ident[:])
            ones_b = const_pool.tile([128, 2], BF, tag="onesb")
            nc.sync.dma_start(ones_b[:], onesB[:])
            ones_f = const_pool.tile([128, 66], F32R, tag="onesf")
            nc.sync.dma_start(ones_f[:], onesF[:].bitcast(F32R))

            wk = []
            for dt in range(NDT):
                w = wk_pool.tile([128, 384], BF, tag="wk", name=f"wk{dt}")
                nc.sync.dma_start(w[:], wqkvT[dt * 128:(dt + 1) * 128, :])
                wk.append(w)

            # qt[b], kt[b]: [128 (2 heads x 64 qk-dims), 1024 tokens]
            qt = [qk_pool.tile([128, S], BF, tag="qk", name=f"qt{b}") for b in range(B)]
            kt = [qk_pool.tile([128, S], BF, tag="qk", name=f"kt{b}") for b in range(B)]
            # v65[b][tb]: [128 tokens, 130] = [V_h0 | ones | V_h1 | ones]
            v65 = [[v65_pool.tile([128, 2 * (DH + 1)], BF, tag="v65",
                                  name=f"v65_{b}_{tb}")
                    for tb in range(NT)] for b in range(B)]

            with (
                tc.tile_pool(name="xt", bufs=B * NDT) as xt_pool,
                tc.tile_pool(name="vt", bufs=B) as vt_pool,
                tc.tile_pool(name="psB", bufs=2, space="PSUM") as psB_pool,
                tc.tile_pool(name="psT", bufs=2, space="PSUM") as psT_pool,
            ):
                xt = []
                for b in range(B):
                    row = []
                    for dt in range(NDT):
                        t = xt_pool.tile([128, S], BF, tag="xt", name=f"xt{b}_{dt}")
                        nc.sync.dma_start(t[:], xT[b, dt * 128:(dt + 1) * 128, :])
                        row.append(t)
                    xt.append(row)

                # ---- QKV^T: psum[qk-dims 128, tokens 1024], accumulate over d
                vt = []
                for b in range(B):
                    for part, dst in ((0, qt[b]), (1, kt[b])):
                        ps = psB_pool.tile([128, S], F32, tag="psB")
                        for dt in range(NDT):
                            for nb in range(2):
                                nc.tensor.matmul(
                                    ps[:, nb * 512:(nb + 1) * 512],
                                    wk[dt][:, part * 128:(part + 1) * 128],
                                    xt[b][dt][:, nb * 512:(nb + 1) * 512],
                                    start=(dt == 0),
                                    stop=(dt == NDT - 1),
                                )
                        nc.vector.tensor_copy(dst[:], ps[:])
                    ps = psB_pool.tile([128, S], F32, tag="psB")
                    for dt in range(NDT):
                        for nb in range(2):
                            nc.tensor.matmul(
                                ps[:, nb * 512:(nb + 1) * 512],
                                wk[dt][:, 256:384],
                                xt[b][dt][:, nb * 512:(nb + 1) * 512],
                                start=(dt == 0),
                                stop=(dt == NDT - 1),
                            )
                    v = vt_pool.tile([128, S], BF, tag="vt", name=f"vt{b}")
                    nc.vector.tensor_copy(v[:], ps[:])
                    vt.append(v)

                # ---- V^T -> V natural tiles with ones columns ----
                for b in range(B):
                    for tb in range(NT):
                        pst = psT_pool.tile([128, 128], BF, tag="psT")
                        nc.tensor.transpose(
                            pst[:], vt[b][:, tb * 128:(tb + 1) * 128], idt[:]
                        )
                        dst = v65[b][tb]
                        for hl in range(2):
                            nc.vector.tensor_copy(
                                dst[:, hl * 65:hl * 65 + 64],
                                pst[:, hl * 64:(hl + 1) * 64],
                            )
                            nc.vector.tensor_copy(
                                dst[:, hl * 65 + 64:hl * 65 + 65],
                                ones_b[:, hl:hl + 1],
                            )

            # ---- attention per (batch, local head) ----
            with (
                tc.tile_pool(name="wo", bufs=NDT) as wo_pool,
                tc.tile_pool(name="expt", bufs=4) as exp_pool,
                tc.tile_pool(name="att", bufs=2) as att_pool,
                tc.tile_pool(name="rcp", bufs=2) as rcp_pool,
                tc.tile_pool(name="bcs", bufs=2) as bcs_pool,
                tc.tile_pool(name="af", bufs=NDT) as af_pool,
                tc.tile_pool(name="osb", bufs=2) as osb_pool,
                tc.tile_pool(name="psC", bufs=2, space="PSUM") as psC_pool,
                tc.tile_pool(name="psA", bufs=2, space="PSUM") as psA_pool,
            ):
                wo = []
                for dt in range(NDT):
                    w = wo_pool.tile([128, D], BF, tag="wo", name=f"wo{dt}")
                    nc.sync.dma_start(w[:], woT[dt * 128:(dt + 1) * 128, :])
                    wo.append(w)

                att = [att_pool.tile([64, B * S], BF, tag="att", name=f"att{hl}")
                       for hl in range(2)]

                for b in range(B):
                    for hl in range(2):
                        p0 = hl * 64
                        pav = psA_pool.tile([65, S], F32, tag="psA")
                        for kb in range(NT):
                            psc = psC_pool.tile([128, S], F32, tag="psC")
                            for nb in range(2):
                                nc.tensor.matmul(
                                    psc[:, nb * 512:(nb + 1) * 512],
                                    kt[b][p0:p0 + 64, kb * 128:(kb + 1) * 128],
                                    qt[b][p0:p0 + 64, nb * 512:(nb + 1) * 512],
                                    start=True,
                                    stop=True,
                                )
                            et = exp_pool.tile([128, S], BF, tag="expt")
                            nc.scalar.activation(et[:], psc[:], EXP)
                            for nb in range(2):
                                nc.tensor.matmul(
                                    pav[:, nb * 512:(nb + 1) * 512],
                                    v65[b][kb][:, hl * 65:(hl + 1) * 65],
                                    et[:, nb * 512:(nb + 1) * 512],
                                    start=(kb == 0),
                                    stop=(kb == NT - 1),
                                )
                        # normalize: f32r reciprocal + PE broadcast + multiply
                        rc = rcp_pool.tile([65, S], F32R, tag="rcp")
                        nc.vector.reciprocal(rc[64:65, :], pav[64:65, :].bitcast(F32R))
                        pbc = psA_pool.tile([64, S], F32, tag="psA")
                        for nb in range(2):
                            nc.tensor.matmul(
                                pbc[:, nb * 512:(nb + 1) * 512],
                                ones_f[64:65, 2:66],
                                rc[64:65, nb * 512:(nb + 1) * 512],
                                start=True,
                                stop=True,
                            )
                        bc = bcs_pool.tile([64, S], F32, tag="bcs")
                        nc.vector.tensor_copy(bc[:], pbc[:])
                        nc.vector.tensor_mul(
                            att[hl][:, b * S:(b + 1) * S], pav[:64, :], bc[:]
                        )

                # ---- AllToAll: head-sharded -> token-sharded ----
                for j in range(N_CORES):
                    col = (j // 4) * S + (j % 4) * 256
                    for hl in range(2):
                        nc.sync.dma_start(
                            a2a_in[it][j * 128 + hl * 64:j * 128 + (hl + 1) * 64, :],
                            att[hl][:, col:col + 256],
                        )
                nc.gpsimd.collective_compute(
                    "AllToAll",
                    mybir.AluOpType.bypass,
                    replica_groups=[list(range(N_CORES))],
                    ins=[a2a_in[it][:]],
                    outs=[a2a_out[it][:]],
                )
                af = []
                for dt in range(NDT):
                    t = af_pool.tile([128, 256], BF, tag="af", name=f"af{dt}")
                    nc.sync.dma_start(t[:], a2a_out[it][dt * 128:(dt + 1) * 128, :])
                    af.append(t)

                # ---- output projection: full W_out, 256-token shard ----
                for eb in range(NDT):
                    po = psC_pool.tile([128, 256], F32, tag="psC")
                    for dt in range(NDT):
                        nc.tensor.matmul(
                            po[:],
                            wo[dt][:, eb * 128:(eb + 1) * 128],
                            af[dt][:],
                            start=(dt == 0),
                            stop=(dt == NDT - 1),
                        )
                    ot = osb_pool.tile([128, 256], F32, tag="osb")
                    nc.vector.tensor_copy(ot[:], po[:])
                    nc.sync.dma_start(outT[eb * 128:(eb + 1) * 128, :], ot[:])

    nc.finalize()
    _nc_cache[iters] = nc
    return nc


def prep_inputs(x, W_qkv, W_out):
    """Build per-core input maps (numpy only)."""
    x = np.asarray(x, dtype=np.float32)
    W_qkv = np.asarray(W_qkv, dtype=np.float32)
    W_out = np.asarray(W_out, dtype=np.float32)

    xT = np.ascontiguousarray(x.transpose(0, 2, 1)).astype(BF16)   # [B, D, S]
    woT = np.ascontiguousarray(W_out.T).astype(BF16)               # [D, D]
    ident = np.eye(128, dtype=BF16)
    onesB = np.ones((128, 2), dtype=BF16)
    onesF = np.ones((128, 66), dtype=np.float32)

    Wr = W_qkv.reshape(3, H, DH, D)
    in_maps = []
    for c in range(N_CORES):
        hs = slice(2 * c, 2 * c + 2)
        wq = Wr[0, hs].reshape(128, D) * SCALE
        wkk = Wr[1, hs].reshape(128, D)
        wv = Wr[2, hs].reshape(128, D)
        wqkvT = np.ascontiguousarray(np.concatenate([wq, wkk, wv], 0).T).astype(BF16)
        in_maps.append({
            "xT": xT,
            "wqkvT": wqkvT,
            "woT": woT,
            "ident": ident,
            "onesB": onesB,
            "onesF": onesF,
        })
    return in_maps


def assemble(results):
    out = np.empty((B, S, D), dtype=np.float32)
    for c in range(N_CORES):
        b, r = c // 4, c % 4
        out[b, r * 256:(r + 1) * 256, :] = results[c]["outT"].T
    return out


def kernel(x, W_qkv, W_out):
    from concourse.bass_utils import run_bass_kernel_spmd

    nc = _build_nc()
    in_maps = prep_inputs(x, W_qkv, W_out)
    res = run_bass_kernel_spmd(nc, in_maps, list(range(N_CORES)))
    return assemble(res.results)
